# revision 1
# baseline (speedup 1.0000x reference)
"""Trainium2 Bass kernel for nn_AverageAttention (B=8, L=2048, D=1024).

Math (per batch b):
    avg[t]  = cumsum(x, axis=t)[t] / (t+1)
    g       = concat([x, avg], -1) @ W_gate.T + b_gate        # (L, 2*D)
    out     = sigmoid(g[:, :D]) * x + sigmoid(g[:, D:]) * avg

Strategy: batch-parallel over 8 NeuronCores (one sequence per core), W_gate
replicated. Everything on-chip runs in transposed (feature-on-partition,
token-on-free) layout so the cumulative sum is a single DVE
tensor_tensor_scan per 128-feature chunk. The gating matmul runs in bf16
(fp32 PSUM accumulation); sigmoid + bias is fused into the PSUM evacuation
on the scalar engine. Host pre/post work is limited to layout transposes,
bf16 weight cast, and constant generation.
"""

from contextlib import ExitStack

import ml_dtypes
import numpy as np

import concourse.bass as bass
import concourse.bass_utils as bass_utils
import concourse.mybir as mybir
import concourse.tile as tile
from concourse import bacc
from concourse._compat import with_exitstack
from concourse.bass import ts

B, L, D = 8, 2048, 1024
NJ = D // 128        # 8 feature chunks of x / avg
NK = 2 * D // 128    # 16 contraction chunks of cat = [x, avg]
NOB = 2 * D // 128   # 16 output-feature blocks of g
import os as _os_mod

TCW = int(_os_mod.environ.get("KTCW", "512"))  # matmul moving free-dim
NTC = L // TCW       # token chunks per 2048

FP32 = mybir.dt.float32
BF16 = mybir.dt.bfloat16

# Contraction chunks ordered as phase 1 produces them (x chunk j, then avg
# chunk NJ+j) so the PE can start before phase 1 finishes. W tiles are laid
# out on the host in this order.
KC_ORDER = []
for _j in range(NJ):
    KC_ORDER.extend([_j, NJ + _j])


@with_exitstack
def _tile_body(
    ctx: ExitStack,
    tc: tile.TileContext,
    n_pairs: int = NJ,
    reps: int = 1,
    two_pass: bool = False,
    ph1: int = 2048,
    gp_mul: bool = False,
    w_stat: bool = True,
):
    nc = tc.nc

    xT = nc.dram_tensor("xT", (NJ, 128, L), FP32, kind="ExternalInput").ap()
    wob = nc.dram_tensor("wob", (NOB, 128, NK, 128), BF16, kind="ExternalInput").ap()
    invd = nc.dram_tensor("invd", (128, L), FP32, kind="ExternalInput").ap()
    biash = nc.dram_tensor("biash", (128, NOB), FP32, kind="ExternalInput").ap()
    avgT = nc.dram_tensor("avgT", (NJ, 128, L), FP32, kind="ExternalOutput").ap()
    gatT = nc.dram_tensor("gatT", (NJ, 128, L), FP32, kind="ExternalOutput").ap()

    cat_pool = ctx.enter_context(tc.tile_pool(name="cat", bufs=NK))
    const_pool = ctx.enter_context(tc.tile_pool(name="const", bufs=1))
    x_pool = ctx.enter_context(tc.tile_pool(name="x", bufs=2))
    cum_pool = ctx.enter_context(tc.tile_pool(name="cum", bufs=2))
    avg_pool = ctx.enter_context(tc.tile_pool(name="avg", bufs=2))
    w_pool = ctx.enter_context(tc.tile_pool(name="w", bufs=3))
    sig_pool = ctx.enter_context(tc.tile_pool(name="sig", bufs=3))
    gat_pool = ctx.enter_context(tc.tile_pool(name="gat", bufs=2))
    psum_pool = ctx.enter_context(
        tc.tile_pool(name="psum", bufs=max(1, 8 * 512 // TCW), space="PSUM")
    )
    if two_pass:
        gx_pool = ctx.enter_context(tc.tile_pool(name="gx", bufs=3 * 2 * NTC))
        gs_pool = ctx.enter_context(tc.tile_pool(name="gs", bufs=3))

    invd_sb = const_pool.tile([128, L], FP32, tag="invd")
    bias_sb = const_pool.tile([128, NOB], FP32, tag="bias")

    # cat[kc] for kc in [0, NJ) is bf16 x; [NJ, NK) is bf16 avg.
    cats = [
        cat_pool.tile([128, L], BF16, tag="cat", name=f"cat{k}") for k in range(NK)
    ]

    def load_pair_w(j, chunks=1):
        # W tiles are stored in KC_ORDER on the host. The first pair loads in
        # chunks so the first Ldweights only waits for a quarter of the tile.
        wt_i = w_pool.tile([128, NK, 128], BF16, name="wt_i", tag="wt_i")
        wt_f = w_pool.tile([128, NK, 128], BF16, name="wt_f", tag="wt_f")
        step = NK // chunks
        for c in range(chunks):
            cs = slice(c * step, (c + 1) * step)
            nc.sync.dma_start(wt_i[:, cs, :], wob[j][:, cs, :])
            nc.sync.dma_start(wt_f[:, cs, :], wob[NJ + j][:, cs, :])
        return wt_i, wt_f

    PH1 = ph1  # phase-1 chunk width (DMA efficiency vs pipeline latency)
    NQ1 = L // PH1

    for _rep in range(reps):
        # Head ordering on the sync HWDGE ring (FIFO per ring): first x
        # chunk, then the first pair's W in chunks, so the first matmul
        # unblocks as early as possible. Constants ride the scalar-engine
        # HWDGE ring so they never queue ahead of inputs.
        xt0 = x_pool.tile([128, L], FP32, name="xt", tag="xt")
        nc.sync.dma_start(xt0[:, ts(0, PH1)], xT[0][:, ts(0, PH1)])
        if _rep == 0:
            nc.scalar.dma_start(bias_sb[:], biash[:])
            nc.scalar.dma_start(invd_sb[:], invd[:])
        w_tiles = {j: load_pair_w(j, chunks=4 if j == 0 else 1)
                   for j in range(min(2, n_pairs))}

        # Phase 1: load xT (two DMA chunks), cast the x half of cat per
        # chunk, one full-width cumsum scan + scale per feature block (DVE
        # instruction count kept minimal — per-op drain overhead on the DVE
        # is large on HW), store avg, cast the avg half of cat.
        for j in range(NJ):
            xt = xt0 if j == 0 else x_pool.tile([128, L], FP32, name="xt", tag="xt")
            ct = cum_pool.tile([128, L], FP32)
            at = avg_pool.tile([128, L], FP32)
            for q in range(NQ1):
                s = ts(q, PH1)
                if not (j == 0 and q == 0):
                    nc.sync.dma_start(xt[:, s], xT[j][:, s])
                nc.gpsimd.tensor_copy(cats[j][:, s], xt[:, s])
            nc.vector.tensor_tensor_scan(
                ct[:],
                xt[:],
                xt[:],
                0.0,
                mybir.AluOpType.add,
                mybir.AluOpType.bypass,
            )
            nc.vector.tensor_mul(at[:], ct[:], invd_sb[:])
            nc.gpsimd.tensor_copy(cats[NJ + j][:], at[:])
            nc.sync.dma_start(avgT[j], at[:])

        # Phase 2: gating matmul g^T = W @ cat^T per 128-row output block;
        # sigmoid(g + bias) fused into PSUM evacuation; elementwise gate
        # combine per (input_gate, forget_gate) pair per token chunk.
        #
        # two_pass: the x half of every accumulation runs as an early pass A
        # (no dependence on the cumsum chain), evacuated to bf16 staging; the
        # avg half accumulates later in pass B and is summed with the staging
        # on the DVE before the sigmoid. Pass A for pair j+2 is emitted ahead
        # of pass B for pair j, giving the PE two pairs of phase-1-independent
        # work to hide the cumsum pipeline.
        XI = [i for i, kc in enumerate(KC_ORDER) if kc < NJ]
        AI = [i for i, kc in enumerate(KC_ORDER) if kc >= NJ]

        def pass_a(wts):
            wt_i, wt_f = wts
            gxs = []
            for tcx in range(NTC):
                s = ts(tcx, TCW)
                for wt in (wt_i, wt_f):
                    ps = psum_pool.tile([128, TCW], FP32, name="ps", tag="ps")
                    for n, i in enumerate(XI):
                        nc.tensor.matmul(
                            ps[:],
                            wt[:, i, :],
                            cats[KC_ORDER[i]][:, s],
                            start=(n == 0),
                            stop=(n == len(XI) - 1),
                        )
                    gx = gx_pool.tile([128, TCW], BF16, name="gx", tag="gx")
                    nc.vector.tensor_copy(gx[:], ps[:])
                    gxs.append(gx)
            return gxs

        for j in range(n_pairs, NJ):
            gt = gat_pool.tile([128, L], FP32, name="gt_stub")
            nc.scalar.copy(gt[:], cats[j][:])
            nc.sync.dma_start(gatT[j], gt[:])

        gx_map = {}
        if two_pass:
            for jj in range(min(2, n_pairs)):
                gx_map[jj] = pass_a(w_tiles[jj])
        for j in range(n_pairs):
            if j + 2 < n_pairs:
                w_tiles[j + 2] = load_pair_w(j + 2)
                if two_pass:
                    gx_map[j + 2] = pass_a(w_tiles[j + 2])
            wt_i, wt_f = w_tiles.pop(j)
            gxs = gx_map.pop(j, None)
            gt = gat_pool.tile([128, L], FP32, name="gt", tag="gt")
            st_i = sig_pool.tile([128, L], FP32, name="st", tag="st")
            st_f = sig_pool.tile([128, L], FP32, name="st", tag="st")
            if w_stat and not two_pass:
                # Weight-stationary order: the 4 token chunks run as 4
                # interleaved PSUM groups so consecutive matmuls share the
                # same stationary weights (amortizes the weight-load path).
                for half, wt, st in ((0, wt_i, st_i), (1, wt_f, st_f)):
                    ob = j + NJ * half
                    pss = [
                        psum_pool.tile([128, TCW], FP32, name="ps", tag="ps")
                        for _ in range(NTC)
                    ]
                    for i, kc in enumerate(KC_ORDER):
                        for tcx in range(NTC):
                            nc.tensor.matmul(
                                pss[tcx][:],
                                wt[:, i, :],
                                cats[kc][:, ts(tcx, TCW)],
                                start=(i == 0),
                                stop=(i == NK - 1),
                            )
                    for tcx in range(NTC):
                        nc.scalar.activation(
                            st[:, ts(tcx, TCW)],
                            pss[tcx][:],
                            mybir.ActivationFunctionType.Sigmoid,
                            bias=bias_sb[:, ob : ob + 1],
                        )
            else:
              for tcx in range(NTC):
                s = ts(tcx, TCW)
                for half, wt, st in ((0, wt_i, st_i), (1, wt_f, st_f)):
                    ob = j + NJ * half
                    ps = psum_pool.tile([128, TCW], FP32, name="ps", tag="ps")
                    if two_pass:
                        for n, i in enumerate(AI):
                            nc.tensor.matmul(
                                ps[:],
                                wt[:, i, :],
                                cats[KC_ORDER[i]][:, s],
                                start=(n == 0),
                                stop=(n == len(AI) - 1),
                            )
                        src = gs_pool.tile([128, TCW], FP32, name="gs", tag="gs")
                        nc.vector.tensor_add(src[:], gxs[tcx * 2 + half][:], ps[:])
                    else:
                        for i, kc in enumerate(KC_ORDER):
                            nc.tensor.matmul(
                                ps[:],
                                wt[:, i, :],
                                cats[kc][:, s],
                                start=(i == 0),
                                stop=(i == NK - 1),
                            )
                        src = ps
                    nc.scalar.activation(
                        st[:, s],
                        src[:],
                        mybir.ActivationFunctionType.Sigmoid,
                        bias=bias_sb[:, ob : ob + 1],
                    )
            # Full-width gate combine (3 ops per pair); optionally move the
            # forget-gate product to the otherwise-idle gpsimd engine. The
            # last pair combines and stores per chunk instead, shortening the
            # serial tail after the final matmul.
            if j == n_pairs - 1:
                for tcx in range(NTC):
                    s = ts(tcx, TCW)
                    nc.vector.tensor_mul(gt[:, s], st_i[:, s], cats[j][:, s])
                    nc.vector.tensor_mul(
                        st_f[:, s], st_f[:, s], cats[NJ + j][:, s]
                    )
                    nc.vector.tensor_add(gt[:, s], gt[:, s], st_f[:, s])
                    nc.sync.dma_start(gatT[j][:, s], gt[:, s])
            else:
                nc.vector.tensor_mul(gt[:], st_i[:], cats[j][:])
                if gp_mul:
                    nc.gpsimd.tensor_mul(st_f[:], st_f[:], cats[NJ + j][:])
                else:
                    nc.vector.tensor_mul(st_f[:], st_f[:], cats[NJ + j][:])
                nc.vector.tensor_add(gt[:], gt[:], st_f[:])
                nc.sync.dma_start(gatT[j], gt[:])


_CACHE: dict = {}


def build_nc(
    n_pairs: int | None = None,
    reps: int | None = None,
    two_pass: bool | None = None,
    ph1: int | None = None,
    gp_mul: bool | None = None,
    w_stat: bool | None = None,
):
    import os as _os

    if n_pairs is None:
        n_pairs = int(_os.environ.get("KN_PAIRS", str(NJ)))
    if reps is None:
        reps = int(_os.environ.get("KREPS", "1"))
    if two_pass is None:
        two_pass = _os.environ.get("KTP", "0") == "1"
    if ph1 is None:
        ph1 = int(_os.environ.get("KPH1", "2048"))
    if gp_mul is None:
        gp_mul = _os.environ.get("KGPMUL", "0") == "1"
    if w_stat is None:
        w_stat = _os.environ.get("KWSTAT", "1") == "1"
    key = ("nc", n_pairs, reps, two_pass, TCW, ph1, gp_mul, w_stat)
    if key not in _CACHE:
        nc = bacc.Bacc(
            "TRN2",
            target_bir_lowering=False,
            debug=False,
            enable_asserts=True,
            num_devices=B,
        )
        with tile.TileContext(nc) as t:
            _tile_body(
                t,
                n_pairs=n_pairs,
                reps=reps,
                two_pass=two_pass,
                ph1=ph1,
                gp_mul=gp_mul,
                w_stat=w_stat,
            )
        nc.compile()
        _CACHE[key] = nc
    return _CACHE[key]


def prep_shared(W_gate: np.ndarray, b_gate: np.ndarray):
    # wob[ob, p, i, o] = W_gate[128*ob + o, 128*KC_ORDER[i] + p]
    wob = np.ascontiguousarray(
        W_gate.astype(np.float32)
        .T.reshape(NK, 128, NOB, 128)
        .transpose(2, 1, 0, 3)[:, :, KC_ORDER, :]
    ).astype(ml_dtypes.bfloat16)
    invd = np.ascontiguousarray(
        np.broadcast_to(
            1.0 / np.arange(1, L + 1, dtype=np.float32)[None, :], (128, L)
        )
    )
    biash = np.ascontiguousarray(
        b_gate.astype(np.float32).reshape(NOB, 128).T
    )
    return wob, invd, biash


def kernel(inputs: np.ndarray, W_gate: np.ndarray, b_gate: np.ndarray, **run_kwargs):
    inputs = np.asarray(inputs, dtype=np.float32)
    W_gate = np.asarray(W_gate, dtype=np.float32)
    b_gate = np.asarray(b_gate, dtype=np.float32)
    assert inputs.shape == (B, L, D)

    wob, invd, biash = prep_shared(W_gate, b_gate)
    in_maps = []
    for c in range(B):
        xT_c = np.ascontiguousarray(inputs[c].T).reshape(NJ, 128, L)
        in_maps.append({"xT": xT_c, "wob": wob, "invd": invd, "biash": biash})

    nc = build_nc()
    res = bass_utils.run_bass_kernel_spmd(
        nc, in_maps, core_ids=list(range(B)), **run_kwargs
    )

    gating = np.empty((B, L, D), dtype=np.float32)
    average = np.empty((B, L, D), dtype=np.float32)
    for c in range(B):
        gating[c] = res.results[c]["gatT"].reshape(D, L).T
        average[c] = res.results[c]["avgT"].reshape(D, L).T
    if run_kwargs:
        _CACHE["last_results"] = res
    return gating, average



# revision 5
# speedup vs baseline: 1.3340x; 1.3340x over previous
"""Trainium2 Bass kernel for nn_AverageAttention (B=8, L=2048, D=1024).

Math (per batch b):
    avg[t]  = cumsum(x, axis=t)[t] / (t+1)
    g       = concat([x, avg], -1) @ W_gate.T + b_gate        # (L, 2*D)
    out     = sigmoid(g[:, :D]) * x + sigmoid(g[:, D:]) * avg

Strategy: batch-parallel over 8 NeuronCores (one sequence per core), W_gate
replicated. On-chip layout is transposed (feature-on-partition,
token-on-free) so the cumulative sum is one DVE tensor_tensor_scan per
128-feature chunk.

The gating matmul runs in fp8-e4m3 with MatmulPerfMode.DoubleRow (two
128-row contraction chunks per instruction; measured ~795ns per
K=2048/N=512/M=128 accumulation group on HW vs ~3950ns for bf16). Contraction
chunk m pairs (x_m, avg_m). The whole W (4MB fp8) lives in SBUF, loaded once
per rep. Accuracy: fp8 operand quantization gives ~1.3e-2 rel on the gating
output (threshold 2e-2); avg path stays fp32-scan/bf16-store (~3e-4).

Outputs cross HBM as bf16 (halves store traffic; ~0.1% rounding), upcast to
fp32 on the host. All DMA rides the otherwise-idle sync (SP) HWDGE ring,
ordered: [W pair0 | invd | bias | x0..x7 | W rest (j-major) | avg stores |
gat stores] so x loads are never head-blocked. Elementwise work is split:
DVE scan + avg-mul + 2/3 of the gate combine, Pool (gpsimd) fp8/bf16 casts +
1/3 combine, Act sigmoid-only.
"""

from contextlib import ExitStack

import ml_dtypes
import numpy as np

import concourse.bass as bass
import concourse.bass_utils as bass_utils
import concourse.mybir as mybir
import concourse.tile as tile
from concourse import bacc
from concourse._compat import with_exitstack
from concourse.bass import ts

B, L, D = 8, 2048, 1024
NJ = D // 128         # 8 feature chunks of x / avg
NOB = 2 * D // 128    # 16 output-feature blocks of g
NP = NJ               # 8 DoubleRow contraction pairs (x_m, avg_m)
TCW = 512             # matmul moving free-dim (1 PSUM bank)
NTC = L // TCW

FP32 = mybir.dt.float32
BF16 = mybir.dt.bfloat16
FP8 = mybir.dt.float8e4

F8NP = ml_dtypes.float8_e4m3
BFNP = ml_dtypes.bfloat16


@with_exitstack
def _tile_body(ctx: ExitStack, tc: tile.TileContext, reps: int = 1):
    nc = tc.nc

    xT = nc.dram_tensor("xT", (NJ, 128, L), FP32, kind="ExternalInput").ap()
    wq = nc.dram_tensor("wq", (128, NOB, NP, 2, 128), FP8, kind="ExternalInput").ap()
    invd = nc.dram_tensor("invd", (128, L), BF16, kind="ExternalInput").ap()
    biash = nc.dram_tensor("biash", (128, NOB), FP32, kind="ExternalInput").ap()
    avgT = nc.dram_tensor("avgT", (NJ, 128, L), BF16, kind="ExternalOutput").ap()
    gatT = nc.dram_tensor("gatT", (NJ, 128, L), BF16, kind="ExternalOutput").ap()

    const_pool = ctx.enter_context(tc.tile_pool(name="const", bufs=1))
    w_pool = ctx.enter_context(tc.tile_pool(name="w", bufs=1))
    cat_pool = ctx.enter_context(tc.tile_pool(name="cat", bufs=NP))
    xbf_pool = ctx.enter_context(tc.tile_pool(name="xbf", bufs=NJ))
    abf_pool = ctx.enter_context(tc.tile_pool(name="abf", bufs=NJ))
    x_pool = ctx.enter_context(tc.tile_pool(name="x", bufs=3))
    ct_pool = ctx.enter_context(tc.tile_pool(name="ct", bufs=2))
    st_pool = ctx.enter_context(tc.tile_pool(name="st", bufs=4))
    gt_pool = ctx.enter_context(tc.tile_pool(name="gt", bufs=2))
    tmp_pool = ctx.enter_context(tc.tile_pool(name="tmp", bufs=1))
    psum_pool = ctx.enter_context(tc.tile_pool(name="psum", bufs=8, space="PSUM"))

    invd_sb = const_pool.tile([128, L], BF16, tag="invd")
    bias_sb = const_pool.tile([128, NOB], FP32, tag="bias")

    for _rep in range(reps):
        w_sb = w_pool.tile([128, NOB, NP, 2, 128], FP8, name="w_sb", tag="w_sb")
        cats = [
            cat_pool.tile([128, 2, L], FP8, tag="cat", name=f"cat{m}")
            for m in range(NP)
        ]
        xbfs = [
            xbf_pool.tile([128, L], BF16, tag="xbf", name=f"xbf{j}") for j in range(NJ)
        ]
        abfs = [
            abf_pool.tile([128, L], BF16, tag="abf", name=f"abf{j}") for j in range(NJ)
        ]

        # --- sync-ring head: first W pair, constants, then all of x ---
        nc.sync.dma_start(w_sb[:, 0], wq[:, 0])
        nc.sync.dma_start(w_sb[:, NJ], wq[:, NJ])
        if _rep == 0:
            nc.sync.dma_start(invd_sb[:], invd[:])
            nc.sync.dma_start(bias_sb[:], biash[:])
        xts = []
        for j in range(NJ):
            xt = x_pool.tile([128, L], FP32, name="xt", tag="xt")
            nc.sync.dma_start(xt[:], xT[j])
            xts.append(xt)
        # remaining W, j-major so pair j's tiles land just before needed
        for j in range(1, NJ):
            nc.sync.dma_start(w_sb[:, j], wq[:, j])
            nc.sync.dma_start(w_sb[:, NJ + j], wq[:, NJ + j])

        # --- phase 1: casts, scan, avg (all chase the x DMAs) ---
        for j in range(NJ):
            xt = xts[j]
            nc.gpsimd.tensor_copy(cats[j][:, 0, :], xt[:])
            nc.gpsimd.tensor_copy(xbfs[j][:], xt[:])
            ct = ct_pool.tile([128, L], FP32, name="ct", tag="ct")
            nc.vector.tensor_tensor_scan(
                ct[:], xt[:], xt[:], 0.0, mybir.AluOpType.add, mybir.AluOpType.bypass
            )
            nc.vector.tensor_mul(abfs[j][:], ct[:], invd_sb[:])
            nc.gpsimd.tensor_copy(cats[j][:, 1, :], abfs[j][:])
            nc.sync.dma_start(avgT[j], abfs[j][:])

        # --- phase 2: DoubleRow fp8 matmul, sigmoid evac, gate combine ---
        for j in range(NJ):
            sts = []
            for ob in (j, NJ + j):
                st = st_pool.tile([128, L], BF16, name="st", tag="st")
                for tcx in range(NTC):
                    s = ts(tcx, TCW)
                    ps = psum_pool.tile([128, TCW], FP32, name="ps", tag="ps")
                    for m in range(NP):
                        nc.tensor.matmul(
                            ps[:],
                            w_sb[:, ob, m],
                            cats[m][:, :, s],
                            start=(m == 0),
                            stop=(m == NP - 1),
                            perf_mode=mybir.MatmulPerfMode.DoubleRow,
                        )
                    nc.scalar.activation(
                        st[:, s],
                        ps[:],
                        mybir.ActivationFunctionType.Sigmoid,
                        bias=bias_sb[:, ob : ob + 1],
                    )
                sts.append(st)
            st_i, st_f = sts
            gt = gt_pool.tile([128, L], BF16, name="gt", tag="gt")
            tmp = tmp_pool.tile([128, L], BF16, name="tmp", tag="tmp")
            if j == NJ - 1:
                # per-chunk tail to shorten the serial path after the last matmul
                for tcx in range(NTC):
                    s = ts(tcx, TCW)
                    nc.vector.tensor_mul(gt[:, s], st_i[:, s], xbfs[j][:, s])
                    nc.gpsimd.tensor_mul(tmp[:, s], st_f[:, s], abfs[j][:, s])
                    nc.vector.tensor_add(gt[:, s], gt[:, s], tmp[:, s])
                    nc.sync.dma_start(gatT[j][:, s], gt[:, s])
            else:
                nc.vector.tensor_mul(gt[:], st_i[:], xbfs[j][:])
                nc.gpsimd.tensor_mul(tmp[:], st_f[:], abfs[j][:])
                nc.vector.tensor_add(gt[:], gt[:], tmp[:])
                nc.sync.dma_start(gatT[j], gt[:])


_CACHE: dict = {}


def build_nc(reps: int | None = None):
    import os as _os

    if reps is None:
        reps = int(_os.environ.get("KREPS", "1"))
    key = ("nc", reps)
    if key not in _CACHE:
        nc = bacc.Bacc(
            "TRN2",
            target_bir_lowering=False,
            debug=False,
            enable_asserts=True,
            num_devices=B,
        )
        with tile.TileContext(nc) as t:
            _tile_body(t, reps=reps)
        nc.compile()
        _CACHE[key] = nc
    return _CACHE[key]


# contraction chunk order: pair m slot 0 = x chunk m, slot 1 = avg chunk m
KC_LIST = [c for m in range(NP) for c in (m, NJ + m)]


def prep_shared(W_gate: np.ndarray, b_gate: np.ndarray):
    # wq[p, ob, m, s, o] = W_gate[128*ob + o, 128*kc(m,s) + p]
    arr = np.ascontiguousarray(W_gate.astype(np.float32)).T.reshape(16, 128, NOB, 128)
    # arr[KC_LIST] is (ms, p, ob, o) -> want (p, ob, ms, o)
    wq = np.ascontiguousarray(arr[KC_LIST].transpose(1, 2, 0, 3)).reshape(
        128, NOB, NP, 2, 128
    ).astype(F8NP)
    invd = np.ascontiguousarray(
        np.broadcast_to(
            (1.0 / np.arange(1, L + 1, dtype=np.float32))[None, :], (128, L)
        )
    ).astype(BFNP)
    biash = np.ascontiguousarray(b_gate.astype(np.float32).reshape(NOB, 128).T)
    return wq, invd, biash


def kernel(inputs: np.ndarray, W_gate: np.ndarray, b_gate: np.ndarray, **run_kwargs):
    inputs = np.asarray(inputs, dtype=np.float32)
    W_gate = np.asarray(W_gate, dtype=np.float32)
    b_gate = np.asarray(b_gate, dtype=np.float32)
    assert inputs.shape == (B, L, D)

    wq, invd, biash = prep_shared(W_gate, b_gate)
    in_maps = []
    for c in range(B):
        xT_c = np.ascontiguousarray(inputs[c].T).reshape(NJ, 128, L)
        in_maps.append({"xT": xT_c, "wq": wq, "invd": invd, "biash": biash})

    nc = build_nc()
    res = bass_utils.run_bass_kernel_spmd(
        nc, in_maps, core_ids=list(range(B)), **run_kwargs
    )

    gating = np.empty((B, L, D), dtype=np.float32)
    average = np.empty((B, L, D), dtype=np.float32)
    for c in range(B):
        gating[c] = res.results[c]["gatT"].astype(np.float32).reshape(D, L).T
        average[c] = res.results[c]["avgT"].astype(np.float32).reshape(D, L).T
    if run_kwargs:
        _CACHE["last_results"] = res
    return gating, average


# revision 27
# speedup vs baseline: 1.6993x; 1.2739x over previous
"""Trainium2 Bass kernel for nn_AverageAttention (B=8, L=2048, D=1024).

Math (per batch b):
    avg[t]  = cumsum(x, axis=t)[t] / (t+1)
    g       = concat([x, avg], -1) @ W_gate.T + b_gate        # (L, 2*D)
    out     = sigmoid(g[:, :D]) * x + sigmoid(g[:, D:]) * avg

Strategy: batch-parallel over 8 NeuronCores (one sequence per core), W_gate
replicated. On-chip layout is transposed (feature-on-partition,
token-on-free) so the cumulative sum is one DVE tensor_tensor_scan per
128-feature chunk.

The gating matmul runs in fp8-e4m3 with MatmulPerfMode.DoubleRow (two
128-row contraction chunks per instruction; measured ~795ns per
K=2048/N=512/M=128 accumulation group on HW vs ~3950ns for bf16). Contraction
chunk m pairs (x_m, avg_m). The whole W (4MB fp8) lives in SBUF, loaded once
per rep. Accuracy: fp8 operand quantization gives ~1.3e-2 rel on the gating
output (threshold 2e-2); avg path stays fp32-scan/bf16-store (~3e-4).

Outputs cross HBM as bf16 (halves store traffic; ~0.1% rounding), upcast to
fp32 on the host. All DMA rides the otherwise-idle sync (SP) HWDGE ring,
ordered: [W pair0 | invd | bias | x0..x7 | W rest (j-major) | avg stores |
gat stores] so x loads are never head-blocked.

Engine placement (HW-measured, not what the CoreSim cost model suggests):
every matmul group needs all 16 contraction chunks, so the kernel is gated
by when the last avg chunk's fp8 cast lands. The DVE (fastest engine) runs
only the scans pre-that-point plus the sigma_f*avg mul and final add of the
gate combine afterwards; Pool (gpsimd, slow per-op but absorbs heavy nominal
load) takes all fp8/bf16 casts, the cumsum*invd mul, and sigma_i*x; Act does
sigmoid evacuation ONLY - it is a single serial engine and any cast placed
ahead of the sigmoids in its in-order queue delays every PSUM evacuation
(that mistake cost +23us). Combine/store emission comes after the whole
phase-1 chain so the in-order queues never head-block the critical path.
"""

from contextlib import ExitStack

import ml_dtypes
import numpy as np

import concourse.bass as bass
import concourse.bass_utils as bass_utils
import concourse.mybir as mybir
import concourse.tile as tile
from concourse import bacc
from concourse._compat import with_exitstack
from concourse.bass import ts

B, L, D = 8, 2048, 1024
NJ = D // 128         # 8 feature chunks of x / avg
NOB = 2 * D // 128    # 16 output-feature blocks of g
NP = NJ               # 8 DoubleRow contraction pairs (x_m, avg_m)
TCW = 512             # matmul moving free-dim (1 PSUM bank)
NTC = L // TCW

FP32 = mybir.dt.float32
BF16 = mybir.dt.bfloat16
FP8 = mybir.dt.float8e4

F8NP = ml_dtypes.float8_e4m3
BFNP = ml_dtypes.bfloat16


@with_exitstack
def _tile_body(
    ctx: ExitStack,
    tc: tile.TileContext,
    reps: int = 1,
    no_mm: bool = False,
    no_act: bool = False,
    no_p1: bool = False,
    add_eng: str = "dve",
    minv_eng: str = "pool",
):
    nc = tc.nc

    xT = nc.dram_tensor("xT", (NJ, 128, L), FP32, kind="ExternalInput").ap()
    wq = nc.dram_tensor("wq", (128, NOB, NP, 2, 128), FP8, kind="ExternalInput").ap()
    invd = nc.dram_tensor("invd", (128, L), BF16, kind="ExternalInput").ap()
    biash = nc.dram_tensor("biash", (128, NOB), FP32, kind="ExternalInput").ap()
    avgT = nc.dram_tensor("avgT", (NJ, 128, L), BF16, kind="ExternalOutput").ap()
    gatT = nc.dram_tensor("gatT", (NJ, 128, L), BF16, kind="ExternalOutput").ap()

    const_pool = ctx.enter_context(tc.tile_pool(name="const", bufs=1))
    w_pool = ctx.enter_context(tc.tile_pool(name="w", bufs=1))
    cat_pool = ctx.enter_context(tc.tile_pool(name="cat", bufs=NP))
    abf_pool = ctx.enter_context(tc.tile_pool(name="abf", bufs=NJ))
    x_pool = ctx.enter_context(tc.tile_pool(name="x", bufs=NJ))
    ct_pool = ctx.enter_context(tc.tile_pool(name="ct", bufs=2))
    st_pool = ctx.enter_context(tc.tile_pool(name="st", bufs=3))
    gt_pool = ctx.enter_context(tc.tile_pool(name="gt", bufs=2))
    tmp_pool = ctx.enter_context(tc.tile_pool(name="tmp", bufs=1))
    psum_pool = ctx.enter_context(tc.tile_pool(name="psum", bufs=8, space="PSUM"))

    invd_sb = const_pool.tile([128, L], BF16, tag="invd")
    bias_sb = const_pool.tile([128, NOB], FP32, tag="bias")

    for _rep in range(reps):
        w_sb = w_pool.tile([128, NOB, NP, 2, 128], FP8, name="w_sb", tag="w_sb")
        cats = [
            cat_pool.tile([128, 2, L], FP8, tag="cat", name=f"cat{m}")
            for m in range(NP)
        ]
        abfs = [
            abf_pool.tile([128, L], BF16, tag="abf", name=f"abf{j}") for j in range(NJ)
        ]

        # --- sync-ring head: first W pair, constants, then all of x ---
        nc.sync.dma_start(w_sb[:, 0], wq[:, 0])
        nc.sync.dma_start(w_sb[:, NJ], wq[:, NJ])
        if _rep == 0:
            nc.sync.dma_start(invd_sb[:], invd[:])
            nc.sync.dma_start(bias_sb[:], biash[:])
        xts = []
        for j in range(NJ):
            xt = x_pool.tile([128, L], FP32, name="xt", tag="xt")
            nc.sync.dma_start(xt[:], xT[j])
            xts.append(xt)
            # Pool: x-half fp8 casts, paced only by the x DMAs
            nc.gpsimd.tensor_copy(cats[j][:, 0, :], xt[:])
        # remaining W, j-major so pair j's tiles land just before needed
        for j in range(1, NJ):
            nc.sync.dma_start(w_sb[:, j], wq[:, j])
            nc.sync.dma_start(w_sb[:, NJ + j], wq[:, NJ + j])

        # --- phase 1: the a8_7 critical chain.
        # DVE runs only scans; Pool only the avg muls; Act casts avg->fp8.
        # Everything else (combine, stores) is emitted after, so the in-order
        # queues never delay the last cat chunk the matmuls wait on.
        for j in range(NJ):
            xt = xts[j]
            if no_p1:
                nc.gpsimd.memset(cats[j][:], 0.25)
                nc.vector.tensor_copy(abfs[j][:], xt[:])
                nc.sync.dma_start(avgT[j], abfs[j][:])
                continue
            ct = ct_pool.tile([128, L], FP32, name="ct", tag="ct")
            nc.vector.tensor_tensor_scan(
                ct[:], xt[:], xt[:], 0.0, mybir.AluOpType.add, mybir.AluOpType.bypass
            )
            if minv_eng == "dve":
                nc.vector.tensor_mul(abfs[j][:], ct[:], invd_sb[:])
            else:
                nc.gpsimd.tensor_mul(abfs[j][:], ct[:], invd_sb[:])
            nc.gpsimd.tensor_copy(cats[j][:, 1, :], abfs[j][:])
            nc.sync.dma_start(avgT[j], abfs[j][:])

        if no_mm:
            for j in range(NJ):
                gt = gt_pool.tile([128, L], BF16, name="gt", tag="gt")
                nc.vector.tensor_mul(gt[:], xts[j][:], abfs[j][:])
                nc.sync.dma_start(gatT[j], gt[:])
            continue

        # --- phase 2: DoubleRow fp8 matmul, sigmoid evac, gate combine ---
        for j in range(NJ):
            sts = []
            for ob in (j, NJ + j):
                st = st_pool.tile([128, L], BF16, name="st", tag="st")
                for tcx in range(NTC):
                    s = ts(tcx, TCW)
                    ps = psum_pool.tile([128, TCW], FP32, name="ps", tag="ps")
                    for m in range(NP):
                        nc.tensor.matmul(
                            ps[:],
                            w_sb[:, ob, m],
                            cats[m][:, :, s],
                            start=(m == 0),
                            stop=(m == NP - 1),
                            perf_mode=mybir.MatmulPerfMode.DoubleRow,
                        )
                    if no_act:
                        nc.scalar.copy(st[:, s], ps[:])
                    else:
                        nc.scalar.activation(
                            st[:, s],
                            ps[:],
                            mybir.ActivationFunctionType.Sigmoid,
                            bias=bias_sb[:, ob : ob + 1],
                        )
                sts.append(st)
            st_i, st_f = sts
            gt = gt_pool.tile([128, L], BF16, name="gt", tag="gt")
            tmp = tmp_pool.tile([128, L], BF16, name="tmp", tag="tmp")
            if j == NJ - 1:
                # per-chunk tail to shorten the serial path after the last matmul
                for tcx in range(NTC):
                    s = ts(tcx, TCW)
                    nc.gpsimd.tensor_mul(tmp[:, s], st_i[:, s], xts[j][:, s])
                    nc.vector.tensor_mul(gt[:, s], st_f[:, s], abfs[j][:, s])
                    nc.vector.tensor_add(gt[:, s], gt[:, s], tmp[:, s])
                    nc.sync.dma_start(gatT[j][:, s], gt[:, s])
            else:
                nc.gpsimd.tensor_mul(tmp[:], st_i[:], xts[j][:])
                nc.vector.tensor_mul(gt[:], st_f[:], abfs[j][:])
                if add_eng == "pool":
                    nc.gpsimd.tensor_add(gt[:], gt[:], tmp[:])
                else:
                    nc.vector.tensor_add(gt[:], gt[:], tmp[:])
                nc.sync.dma_start(gatT[j], gt[:])


_CACHE: dict = {}


def build_nc(reps: int | None = None):
    import os as _os

    if reps is None:
        reps = int(_os.environ.get("KREPS", "1"))
    no_mm = _os.environ.get("KNOMM", "0") == "1"
    no_act = _os.environ.get("KNOACT", "0") == "1"
    no_p1 = _os.environ.get("KNOP1", "0") == "1"
    add_eng = _os.environ.get("KADD", "dve")
    minv_eng = _os.environ.get("KMINV", "pool")
    key = ("nc", reps, no_mm, no_act, no_p1, add_eng, minv_eng)
    if key not in _CACHE:
        nc = bacc.Bacc(
            "TRN2",
            target_bir_lowering=False,
            debug=False,
            enable_asserts=True,
            num_devices=B,
        )
        with tile.TileContext(nc) as t:
            _tile_body(
                t, reps=reps, no_mm=no_mm, no_act=no_act, no_p1=no_p1,
                add_eng=add_eng, minv_eng=minv_eng,
            )
        nc.compile()
        _CACHE[key] = nc
    return _CACHE[key]


# contraction chunk order: pair m slot 0 = x chunk m, slot 1 = avg chunk m
KC_LIST = [c for m in range(NP) for c in (m, NJ + m)]


def prep_shared(W_gate: np.ndarray, b_gate: np.ndarray):
    # wq[p, ob, m, s, o] = W_gate[128*ob + o, 128*kc(m,s) + p]
    arr = np.ascontiguousarray(W_gate.astype(np.float32)).T.reshape(16, 128, NOB, 128)
    # arr[KC_LIST] is (ms, p, ob, o) -> want (p, ob, ms, o)
    wq = np.ascontiguousarray(arr[KC_LIST].transpose(1, 2, 0, 3)).reshape(
        128, NOB, NP, 2, 128
    ).astype(F8NP)
    invd = np.ascontiguousarray(
        np.broadcast_to(
            (1.0 / np.arange(1, L + 1, dtype=np.float32))[None, :], (128, L)
        )
    ).astype(BFNP)
    biash = np.ascontiguousarray(b_gate.astype(np.float32).reshape(NOB, 128).T)
    return wq, invd, biash


def kernel(inputs: np.ndarray, W_gate: np.ndarray, b_gate: np.ndarray, **run_kwargs):
    inputs = np.asarray(inputs, dtype=np.float32)
    W_gate = np.asarray(W_gate, dtype=np.float32)
    b_gate = np.asarray(b_gate, dtype=np.float32)
    assert inputs.shape == (B, L, D)

    wq, invd, biash = prep_shared(W_gate, b_gate)
    in_maps = []
    for c in range(B):
        xT_c = np.ascontiguousarray(inputs[c].T).reshape(NJ, 128, L)
        in_maps.append({"xT": xT_c, "wq": wq, "invd": invd, "biash": biash})

    nc = build_nc()
    res = bass_utils.run_bass_kernel_spmd(
        nc, in_maps, core_ids=list(range(B)), **run_kwargs
    )

    gating = np.empty((B, L, D), dtype=np.float32)
    average = np.empty((B, L, D), dtype=np.float32)
    for c in range(B):
        gating[c] = res.results[c]["gatT"].astype(np.float32).reshape(D, L).T
        average[c] = res.results[c]["avgT"].astype(np.float32).reshape(D, L).T
    if run_kwargs:
        _CACHE["last_results"] = res
    return gating, average
